# revision 5
# baseline (speedup 1.0000x reference)
"""LSTM decoder kernel for Trainium2, 8 NeuronCores.

Strategy: data-parallel over batch (32 rows/core, no collectives).
Per-core matmuls are batch-major with 4x column tiling (each 32-wide
col-tile computes a different 256-col window of the gate dim, writing
PSUM partitions 32j:32j+32). The recurrent h @ W_hh.T runs in bf16.

x_proj (constant across timesteps) is injected exactly in fp32 by the
scalar engine writing it straight into the gates PSUM tiles each step;
the gate matmuls then accumulate on top with start=False. This works
because the PSUM has_written bits are set once by warmup matmuls and
never cleared (no start=True afterwards), so accumulating matmuls add
onto whatever the scalar engine wrote.

The gate dim is computed in four phases i, g, f, o placed in three
separate PSUM banks (ig / f / o) so the activation + cell-update chain
pipelines with the matmul stream: sigmoid(i), tanh(g) and i*g run while
the f matmuls stream; c = f*c + i*g and tanh(c) run while the o matmuls
stream. Only sigmoid(o), h = o*tanh(c), the h transposes and their SBUF
copies remain on the per-step critical path.

Layout (per core, batch b in [0,32), col-tile j in [0,4)):
  gate G at psum_G[32j+b, w] = pre_G[b, 256j+w]  (ig tile: i at 0:256,
  g at 256:512; f/o tiles at 0:256)
  c/h tiles [128,256]: [32j+b, w] = state[b, 256j+w]
  hT chunks k: (T1 if k even else T2)[:, 32*(k//2)+ :32]
  W device cols (per 128-chunk k): [i | g | f | o] blocks of 1024, each
  block = 4 strips of 256.
"""
import numpy as np
import ml_dtypes

import concourse.bass as bass
import concourse.mybir as mybir
import concourse.tile as tile
from concourse import bacc
from concourse import bass_utils

B, H, O, T, NCORES = 256, 1024, 512, 128, 8
BL = B // NCORES          # 32 batch rows per core
BF16 = mybir.dt.bfloat16
F32 = mybir.dt.float32

_CACHE = {}


def _emit_phase_mms(nc, out_ps, out_off, w_sb, gate_blk, t1, t2, stop=False):
    """One gate phase: 32 matmuls (8 contraction chunks x 4 col strips)."""
    for k in range(8):
        tt = t1 if k % 2 == 0 else t2
        stat = tt[:, 32 * (k // 2):32 * (k // 2) + 32]
        for j in range(4):
            nc.tensor.matmul(
                out_ps[32 * j:32 * (j + 1), out_off:out_off + 256],
                stat,
                w_sb[k][:, 1024 * gate_blk + 256 * j:1024 * gate_blk + 256 * (j + 1)],
                start=False,
                stop=(stop and k == 7 and j == 3),
                tile_position=(0, 32 * j),
                skip_group_check=True,
            )


def _emit_y_mms(nc, y_ps, wl_sb, t1, t2):
    for k in range(8):
        tt = t1 if k % 2 == 0 else t2
        stat = tt[:, 32 * (k // 2):32 * (k // 2) + 32]
        for j in range(4):
            nc.tensor.matmul(
                y_ps[32 * j:32 * (j + 1), 0:128],
                stat,
                wl_sb[:, 512 * k + 128 * j:512 * k + 128 * j + 128],
                start=(k == 0),
                stop=(k == 7 and j == 3),
                tile_position=(0, 32 * j),
                skip_group_check=True,
            )


def _build(steps=T):
    nc = bacc.Bacc("TRN2", target_bir_lowering=False, debug=False,
                   num_devices=NCORES)
    w_d = nc.dram_tensor("W", [128, 8 * 4096], BF16, kind="ExternalInput").ap()
    wl_d = nc.dram_tensor("Wl", [128, 4096], BF16, kind="ExternalInput").ap()
    xpp_d = nc.dram_tensor("xpp", [128, 1024], F32, kind="ExternalInput").ap()
    eyeb_d = nc.dram_tensor("eyeb", [128, 128], BF16, kind="ExternalInput").ap()
    y_d = nc.dram_tensor("y", [steps, 128, 128], F32,
                         kind="ExternalOutput").ap()

    ACT = mybir.ActivationFunctionType
    mult = mybir.AluOpType.mult
    addop = mybir.AluOpType.add

    with tile.TileContext(nc) as tc:
        with tc.tile_pool(name="stat", bufs=1) as statp, \
             tc.tile_pool(name="sb", bufs=2) as sb, \
             tc.tile_pool(name="psg", bufs=1, space="PSUM") as psg, \
             tc.tile_pool(name="pst", bufs=1, space="PSUM") as pst:
            w_sb = []
            for k in range(8):
                wk = statp.tile([128, 4096], BF16, tag=f"W{k}", name=f"W{k}")
                nc.sync.dma_start(wk[:], w_d[:, 4096 * k:4096 * (k + 1)])
                w_sb.append(wk)
            wl_sb = statp.tile([128, 4096], BF16, tag="Wl", name="Wl")
            nc.sync.dma_start(wl_sb[:], wl_d)
            xpp_sb = statp.tile([128, 1024], F32, tag="xpp", name="xpp")
            nc.sync.dma_start(xpp_sb[:], xpp_d)
            eyeb = statp.tile([128, 128], BF16, tag="eyeb", name="eyeb")
            nc.sync.dma_start(eyeb[:], eyeb_d)
            c_sb = statp.tile([128, 256], F32, tag="c", name="c")
            nc.gpsimd.memset(c_sb[:], 0.0)

            # Gate PSUM tiles: full-bank each (collision isolation), two
            # step-alternating buffers of (ig, f, o).
            gbuf = []
            for s in "AB":
                ig = psg.tile([128, 512], F32, tag=f"ig{s}", name=f"ig{s}")
                fT = psg.tile([128, 512], F32, tag=f"f{s}", name=f"f{s}")
                oT = psg.tile([128, 512], F32, tag=f"o{s}", name=f"o{s}")
                gbuf.append((ig, fT, oT))
            y_ps = pst.tile([128, 512], F32, tag="y", name="y_ps")
            tp = pst.tile([128, 1024], BF16, tag="tp", name="tp")

            # Warmup: set has_written bits everywhere the gate matmuls and
            # x_proj preloads will write (values immediately overwritten).
            for ig, fT, oT in gbuf:
                for tile_, width in ((ig, 512), (fT, 256), (oT, 256)):
                    nc.tensor.matmul(
                        tile_[:, 0:width], eyeb[:], w_sb[0][:, 0:width],
                        start=True, stop=True, skip_group_check=True)
            # Preload x_proj for steps 0 and 1.
            for ig, fT, oT in gbuf:
                nc.scalar.activation(ig[:, 0:512], xpp_sb[:, 0:512], ACT.Copy)
                nc.scalar.activation(fT[:, 0:256], xpp_sb[:, 512:768], ACT.Copy)
                nc.scalar.activation(oT[:, 0:256], xpp_sb[:, 768:1024], ACT.Copy)

            t1_prev = t2_prev = None
            for t in range(steps):
                ig, fT, oT = gbuf[t % 2]
                if t > 0:
                    _emit_phase_mms(nc, ig, 0, w_sb, 0, t1_prev, t2_prev)
                    _emit_phase_mms(nc, ig, 256, w_sb, 1, t1_prev, t2_prev,
                                    stop=True)
                    _emit_phase_mms(nc, fT, 0, w_sb, 2, t1_prev, t2_prev,
                                    stop=True)
                    _emit_phase_mms(nc, oT, 0, w_sb, 3, t1_prev, t2_prev,
                                    stop=True)

                sig_i = sb.tile([128, 256], F32, tag="sig_i", name="sig_i")
                nc.scalar.activation(sig_i[:], ig[:, 0:256], ACT.Sigmoid)
                gt = sb.tile([128, 256], F32, tag="gt", name="gt")
                nc.scalar.activation(gt[:], ig[:, 256:512], ACT.Tanh)
                sig_f = sb.tile([128, 256], F32, tag="sig_f", name="sig_f")
                nc.scalar.activation(sig_f[:], fT[:, 0:256], ACT.Sigmoid)
                th = sb.tile([128, 256], BF16, tag="th", name="th")
                sig_o = sb.tile([128, 256], BF16, tag="sig_o", name="sig_o")
                nc.scalar.activation(sig_o[:], oT[:, 0:256], ACT.Sigmoid)

                tmp = sb.tile([128, 256], F32, tag="tmp", name="tmp")
                nc.vector.tensor_tensor(tmp[:], sig_i[:], gt[:], mult)
                nc.vector.tensor_tensor(c_sb[:], sig_f[:], c_sb[:], mult)
                nc.vector.tensor_tensor(c_sb[:], c_sb[:], tmp[:], addop)
                nc.scalar.activation(th[:], c_sb[:], ACT.Tanh)
                h_sb = sb.tile([128, 256], BF16, tag="h", name="h_sb")
                nc.vector.tensor_tensor(h_sb[:], sig_o[:], th[:], mult)

                nc.tensor.transpose(tp[:, 0:128], h_sb[:, 0:128], eyeb[:])
                nc.tensor.transpose(tp[:, 128:256], h_sb[:, 128:256], eyeb[:])
                t1 = sb.tile([128, 128], BF16, tag="t1", name="t1")
                nc.vector.tensor_copy(t1[:], tp[:, 0:128])
                t2 = sb.tile([128, 128], BF16, tag="t2", name="t2")
                nc.vector.tensor_copy(t2[:], tp[:, 128:256])

                if t > 0:
                    _emit_y_mms(nc, y_ps, wl_sb, t1_prev, t2_prev)
                    y_sb = sb.tile([128, 128], F32, tag="ysb", name="y_sb")
                    nc.vector.tensor_copy(y_sb[:], y_ps[:, 0:128])
                    nc.sync.dma_start(y_d[t - 1], y_sb[:])

                # Preload x_proj into this buffer for step t+2 (scheduler
                # slots these into ACT idle windows after each phase's reads).
                if t < steps - 2:
                    nc.scalar.activation(ig[:, 0:512], xpp_sb[:, 0:512],
                                         ACT.Copy)
                    nc.scalar.activation(fT[:, 0:256], xpp_sb[:, 512:768],
                                         ACT.Copy)
                    nc.scalar.activation(oT[:, 0:256], xpp_sb[:, 768:1024],
                                         ACT.Copy)

                t1_prev, t2_prev = t1, t2

            _emit_y_mms(nc, y_ps, wl_sb, t1_prev, t2_prev)
            y_sb = sb.tile([128, 128], F32, tag="ysb", name="y_sbf")
            nc.vector.tensor_copy(y_sb[:], y_ps[:, 0:128])
            nc.sync.dma_start(y_d[steps - 1], y_sb[:])

    nc.compile()
    return nc


def _colmap():
    """Device gate-col d (per k-chunk) -> original gate column.

    Device blocks are [i | g | f | o] of 1024 each (4 strips x 256);
    original gate order is i, f, g, o."""
    og = [0, 2, 1, 3]
    m = np.empty(4096, np.int64)
    ar = np.arange(256)
    for G in range(4):
        for j in range(4):
            m[1024 * G + 256 * j:1024 * G + 256 * (j + 1)] = \
                og[G] * 1024 + 256 * j + ar
    return m


def _prep_inputs(C, W_ih, W_hh, b_ih, b_hh, W_lin):
    xp = np.asarray(C, np.float32) @ np.asarray(W_ih, np.float32).T
    xp = xp + np.asarray(b_ih, np.float32) + np.asarray(b_hh, np.float32)
    cm = _colmap()
    w_perm = np.asarray(W_hh, np.float32).T[:, cm]
    w_dev = np.ascontiguousarray(
        w_perm.reshape(8, 128, 4096)
        .transpose(1, 0, 2).reshape(128, 8 * 4096)).astype(ml_dtypes.bfloat16)
    wl_dev = np.ascontiguousarray(
        np.asarray(W_lin, np.float32).T.reshape(8, 128, 512)
        .transpose(1, 0, 2).reshape(128, 4096)).astype(ml_dtypes.bfloat16)
    eyeb = np.eye(128, dtype=ml_dtypes.bfloat16)
    in_maps = []
    for c in range(NCORES):
        xpb = xp[BL * c:BL * (c + 1)][:, cm]   # [32, 4096] in device col order
        # xpp[32j+b, 256*G + w] = xpb[b, 1024*G + 256*j + w]
        xpp = np.ascontiguousarray(
            xpb.reshape(32, 4, 4, 256).transpose(2, 0, 1, 3).reshape(128, 1024))
        in_maps.append({"W": w_dev, "Wl": wl_dev, "xpp": xpp, "eyeb": eyeb})
    return in_maps


def kernel(C, W_ih, W_hh, b_ih, b_hh, W_lin, b_lin, max_seq_len):
    assert int(max_seq_len) == T and C.shape == (B, H)
    if "nc" not in _CACHE:
        _CACHE["nc"] = _build()
    nc = _CACHE["nc"]
    in_maps = _prep_inputs(C, W_ih, W_hh, b_ih, b_hh, W_lin)
    try:
        res = bass_utils.run_bass_kernel_spmd(
            nc, in_maps, core_ids=list(range(NCORES)))
    except Exception:
        # transient NRT faults have been observed on this fabric; retry once
        res = bass_utils.run_bass_kernel_spmd(
            nc, in_maps, core_ids=list(range(NCORES)))
    out = np.empty((T, B, O), np.float32)
    blin = np.asarray(b_lin, np.float32)
    for c in range(NCORES):
        yc = res.results[c]["y"]          # [T, 128, 128]
        out[:, BL * c:BL * (c + 1), :] = (
            yc.reshape(T, 4, BL, 128).transpose(0, 2, 1, 3).reshape(T, BL, O)
            + blin)
    return out


# revision 6
# speedup vs baseline: 1085.4470x; 1085.4470x over previous
"""LSTM decoder kernel for Trainium2, 8 NeuronCores.

Strategy: data-parallel over batch (32 rows/core, no collectives).
Per-core matmuls are batch-major with 4x column tiling (each 32-wide
col-tile computes a different 256-col window of the gate dim, writing
PSUM partitions 32j:32j+32). The recurrent h @ W_hh.T runs in bf16.

x_proj (constant across timesteps) is injected exactly in fp32 by the
scalar engine writing it straight into the gates PSUM tiles each step;
the gate matmuls then accumulate on top with start=False. This works
because the PSUM has_written bits are set once by warmup matmuls and
never cleared (no start=True afterwards), so accumulating matmuls add
onto whatever the scalar engine wrote.

The gate dim is computed in four phases i, g, f, o placed in three
separate PSUM banks (ig / f / o) so the activation + cell-update chain
pipelines with the matmul stream: sigmoid(i), tanh(g) and i*g run while
the f matmuls stream; c = f*c + i*g and tanh(c) run while the o matmuls
stream. Only sigmoid(o), h = o*tanh(c), the h transposes and their SBUF
copies remain on the per-step critical path.

Layout (per core, batch b in [0,32), col-tile j in [0,4)):
  gate G at psum_G[32j+b, w] = pre_G[b, 256j+w]  (ig tile: i at 0:256,
  g at 256:512; f/o tiles at 0:256)
  c/h tiles [128,256]: [32j+b, w] = state[b, 256j+w]
  hT chunks k: (T1 if k even else T2)[:, 32*(k//2)+ :32]
  W device cols (per 128-chunk k): [i | g | f | o] blocks of 1024, each
  block = 4 strips of 256.
"""
import numpy as np
import ml_dtypes

import concourse.bass as bass
import concourse.mybir as mybir
import concourse.tile as tile
from concourse import bacc
from concourse import bass_utils

B, H, O, T, NCORES = 256, 1024, 512, 128, 8
BL = B // NCORES          # 32 batch rows per core
BF16 = mybir.dt.bfloat16
F32 = mybir.dt.float32

_CACHE = {}


def _emit_phase_mms(nc, out_ps, w_sb, col_off, width, stop=False):
    """One gate phase: 32 matmuls (8 contraction chunks x 4 col strips).

    W device cols [col_off + width*j + w] land at psum[32j+b, w]."""
    for k in range(8):
        tt = w_sb[8] if k % 2 == 0 else w_sb[9]
        stat = tt[:, 32 * (k // 2):32 * (k // 2) + 32]
        for j in range(4):
            nc.tensor.matmul(
                out_ps[32 * j:32 * (j + 1), 0:width],
                stat,
                w_sb[k][:, col_off + width * j:col_off + width * (j + 1)],
                start=False,
                stop=(stop and k == 7 and j == 3),
                tile_position=(0, 32 * j),
                skip_group_check=True,
            )


def _emit_y_mms(nc, y_ps, wl_sb, t1, t2):
    for k in range(8):
        tt = t1 if k % 2 == 0 else t2
        stat = tt[:, 32 * (k // 2):32 * (k // 2) + 32]
        for j in range(4):
            nc.tensor.matmul(
                y_ps[32 * j:32 * (j + 1), 0:128],
                stat,
                wl_sb[:, 512 * k + 128 * j:512 * k + 128 * j + 128],
                start=(k == 0),
                stop=(k == 7 and j == 3),
                tile_position=(0, 32 * j),
                skip_group_check=True,
            )


def _build(steps=T):
    nc = bacc.Bacc("TRN2", target_bir_lowering=False, debug=False,
                   num_devices=NCORES)
    w_d = nc.dram_tensor("W", [128, 8 * 4096], BF16, kind="ExternalInput").ap()
    wl_d = nc.dram_tensor("Wl", [128, 4096], BF16, kind="ExternalInput").ap()
    xpp_d = nc.dram_tensor("xpp", [128, 1024], F32, kind="ExternalInput").ap()
    eyeb_d = nc.dram_tensor("eyeb", [128, 128], BF16, kind="ExternalInput").ap()
    y_d = nc.dram_tensor("y", [steps, 128, 128], F32,
                         kind="ExternalOutput").ap()

    ACT = mybir.ActivationFunctionType
    mult = mybir.AluOpType.mult
    addop = mybir.AluOpType.add

    with tile.TileContext(nc) as tc:
        with tc.tile_pool(name="stat", bufs=1) as statp, \
             tc.tile_pool(name="sb", bufs=2) as sb, \
             tc.tile_pool(name="psg", bufs=1, space="PSUM") as psg, \
             tc.tile_pool(name="pst", bufs=1, space="PSUM") as pst:
            w_sb = []
            for k in range(8):
                wk = statp.tile([128, 4096], BF16, tag=f"W{k}", name=f"W{k}")
                nc.sync.dma_start(wk[:], w_d[:, 4096 * k:4096 * (k + 1)])
                w_sb.append(wk)
            wl_sb = statp.tile([128, 4096], BF16, tag="Wl", name="Wl")
            nc.sync.dma_start(wl_sb[:], wl_d)
            xpp_sb = statp.tile([128, 1024], F32, tag="xpp", name="xpp")
            nc.sync.dma_start(xpp_sb[:], xpp_d)
            eyeb = statp.tile([128, 128], BF16, tag="eyeb", name="eyeb")
            nc.sync.dma_start(eyeb[:], eyeb_d)
            c_sb = statp.tile([128, 256], F32, tag="c", name="c")
            nc.gpsimd.memset(c_sb[:], 0.0)

            # Gate PSUM tiles: full-bank each (collision isolation), two
            # step-alternating buffers of (ig, f, o).
            gbuf = []
            for s in "AB":
                ig = psg.tile([128, 512], F32, tag=f"ig{s}", name=f"ig{s}")
                fT = psg.tile([128, 512], F32, tag=f"f{s}", name=f"f{s}")
                oT = psg.tile([128, 512], F32, tag=f"o{s}", name=f"o{s}")
                gbuf.append((ig, fT, oT))
            y_ps = pst.tile([128, 512], F32, tag="y", name="y_ps")
            tp = pst.tile([128, 1024], BF16, tag="tp", name="tp")

            # Warmup: set has_written bits everywhere the gate matmuls and
            # x_proj preloads will write (values immediately overwritten).
            for ig, fT, oT in gbuf:
                for tile_, width in ((ig, 512), (fT, 256), (oT, 256)):
                    nc.tensor.matmul(
                        tile_[:, 0:width], eyeb[:], w_sb[0][:, 0:width],
                        start=True, stop=True, skip_group_check=True)
            # Preload x_proj for steps 0 and 1.
            for ig, fT, oT in gbuf:
                nc.scalar.activation(ig[:, 0:512], xpp_sb[:, 0:512], ACT.Copy)
                nc.scalar.activation(fT[:, 0:256], xpp_sb[:, 512:768], ACT.Copy)
                nc.scalar.activation(oT[:, 0:256], xpp_sb[:, 768:1024], ACT.Copy)

            t1_prev = t2_prev = None
            for t in range(steps):
                ig, fT, oT = gbuf[t % 2]
                if t > 0:
                    _emit_phase_mms(nc, ig, 0, w_sb, 0, t1_prev, t2_prev)
                    _emit_phase_mms(nc, ig, 256, w_sb, 1, t1_prev, t2_prev,
                                    stop=True)
                    _emit_phase_mms(nc, fT, 0, w_sb, 2, t1_prev, t2_prev,
                                    stop=True)
                    _emit_phase_mms(nc, oT, 0, w_sb, 3, t1_prev, t2_prev,
                                    stop=True)

                sig_i = sb.tile([128, 256], F32, tag="sig_i", name="sig_i")
                nc.scalar.activation(sig_i[:], ig[:, 0:256], ACT.Sigmoid)
                gt = sb.tile([128, 256], F32, tag="gt", name="gt")
                nc.scalar.activation(gt[:], ig[:, 256:512], ACT.Tanh)
                sig_f = sb.tile([128, 256], F32, tag="sig_f", name="sig_f")
                nc.scalar.activation(sig_f[:], fT[:, 0:256], ACT.Sigmoid)
                th = sb.tile([128, 256], BF16, tag="th", name="th")
                sig_o = sb.tile([128, 256], BF16, tag="sig_o", name="sig_o")
                nc.scalar.activation(sig_o[:], oT[:, 0:256], ACT.Sigmoid)

                tmp = sb.tile([128, 256], F32, tag="tmp", name="tmp")
                nc.vector.tensor_tensor(tmp[:], sig_i[:], gt[:], mult)
                nc.vector.tensor_tensor(c_sb[:], sig_f[:], c_sb[:], mult)
                nc.vector.tensor_tensor(c_sb[:], c_sb[:], tmp[:], addop)
                nc.scalar.activation(th[:], c_sb[:], ACT.Tanh)
                h_sb = sb.tile([128, 256], BF16, tag="h", name="h_sb")
                nc.vector.tensor_tensor(h_sb[:], sig_o[:], th[:], mult)

                nc.tensor.transpose(tp[:, 0:128], h_sb[:, 0:128], eyeb[:])
                nc.tensor.transpose(tp[:, 128:256], h_sb[:, 128:256], eyeb[:])
                t1 = sb.tile([128, 128], BF16, tag="t1", name="t1")
                nc.vector.tensor_copy(t1[:], tp[:, 0:128])
                t2 = sb.tile([128, 128], BF16, tag="t2", name="t2")
                nc.vector.tensor_copy(t2[:], tp[:, 128:256])

                if t > 0:
                    _emit_y_mms(nc, y_ps, wl_sb, t1_prev, t2_prev)
                    y_sb = sb.tile([128, 128], F32, tag="ysb", name="y_sb")
                    nc.vector.tensor_copy(y_sb[:], y_ps[:, 0:128])
                    nc.sync.dma_start(y_d[t - 1], y_sb[:])

                # Preload x_proj into this buffer for step t+2 (scheduler
                # slots these into ACT idle windows after each phase's reads).
                if t < steps - 2:
                    nc.scalar.activation(ig[:, 0:512], xpp_sb[:, 0:512],
                                         ACT.Copy)
                    nc.scalar.activation(fT[:, 0:256], xpp_sb[:, 512:768],
                                         ACT.Copy)
                    nc.scalar.activation(oT[:, 0:256], xpp_sb[:, 768:1024],
                                         ACT.Copy)

                t1_prev, t2_prev = t1, t2

            _emit_y_mms(nc, y_ps, wl_sb, t1_prev, t2_prev)
            y_sb = sb.tile([128, 128], F32, tag="ysb", name="y_sbf")
            nc.vector.tensor_copy(y_sb[:], y_ps[:, 0:128])
            nc.sync.dma_start(y_d[steps - 1], y_sb[:])

    nc.compile()
    return nc


def _colmap():
    """Device gate-col d (per k-chunk) -> original gate column.

    Device blocks are [i | g | f | o] of 1024 each (4 strips x 256);
    original gate order is i, f, g, o."""
    og = [0, 2, 1, 3]
    m = np.empty(4096, np.int64)
    ar = np.arange(256)
    for G in range(4):
        for j in range(4):
            m[1024 * G + 256 * j:1024 * G + 256 * (j + 1)] = \
                og[G] * 1024 + 256 * j + ar
    return m


def _prep_inputs(C, W_ih, W_hh, b_ih, b_hh, W_lin):
    xp = np.asarray(C, np.float32) @ np.asarray(W_ih, np.float32).T
    xp = xp + np.asarray(b_ih, np.float32) + np.asarray(b_hh, np.float32)
    cm = _colmap()
    w_perm = np.asarray(W_hh, np.float32).T[:, cm]
    w_dev = np.ascontiguousarray(
        w_perm.reshape(8, 128, 4096)
        .transpose(1, 0, 2).reshape(128, 8 * 4096)).astype(ml_dtypes.bfloat16)
    wl_dev = np.ascontiguousarray(
        np.asarray(W_lin, np.float32).T.reshape(8, 128, 512)
        .transpose(1, 0, 2).reshape(128, 4096)).astype(ml_dtypes.bfloat16)
    eyeb = np.eye(128, dtype=ml_dtypes.bfloat16)
    in_maps = []
    for c in range(NCORES):
        xpb = xp[BL * c:BL * (c + 1)][:, cm]   # [32, 4096] in device col order
        # xpp[32j+b, 256*G + w] = xpb[b, 1024*G + 256*j + w]
        xpp = np.ascontiguousarray(
            xpb.reshape(32, 4, 4, 256).transpose(2, 0, 1, 3).reshape(128, 1024))
        in_maps.append({"W": w_dev, "Wl": wl_dev, "xpp": xpp, "eyeb": eyeb})
    return in_maps


def kernel(C, W_ih, W_hh, b_ih, b_hh, W_lin, b_lin, max_seq_len):
    assert int(max_seq_len) == T and C.shape == (B, H)
    if "nc" not in _CACHE:
        _CACHE["nc"] = _build()
    nc = _CACHE["nc"]
    in_maps = _prep_inputs(C, W_ih, W_hh, b_ih, b_hh, W_lin)
    try:
        res = bass_utils.run_bass_kernel_spmd(
            nc, in_maps, core_ids=list(range(NCORES)))
    except Exception:
        # transient NRT faults have been observed on this fabric; retry once
        res = bass_utils.run_bass_kernel_spmd(
            nc, in_maps, core_ids=list(range(NCORES)))
    out = np.empty((T, B, O), np.float32)
    blin = np.asarray(b_lin, np.float32)
    for c in range(NCORES):
        yc = res.results[c]["y"]          # [T, 128, 128]
        out[:, BL * c:BL * (c + 1), :] = (
            yc.reshape(T, 4, BL, 128).transpose(0, 2, 1, 3).reshape(T, BL, O)
            + blin)
    return out
